# revision 21
# baseline (speedup 1.0000x reference)
"""Trainium2 Bass kernel for nn_DecoderLayer (temporal self-attn decoder layer).

Sharding: data-parallel over batch B=8, one batch per NeuronCore. Each core
runs an identical program on its batch slice; weights are replicated.

Per-core dataflow (feature-major activations X^T [D=256 -> 2x128 partitions,
tokens], tokens ordered hw-major: col = hw*T + t). DRAM scratch is split into
per-chunk tiles (CH=384 tokens = 4 hw for x-side, 2 hw for memory-side) so the
Tile scheduler can pipeline phases against each other instead of serializing
on whole-tensor dependencies.
"""
import numpy as np

D, NH, HD, FF = 256, 8, 32, 1024
B, T_OUT, T_IN, HW = 8, 96, 192, 64
NTOK = T_OUT * HW        # 6144
NTOKM = T_IN * HW        # 12288
CH = 384                 # 4 hw (x-side) / 2 hw (memory-side) per chunk
NCH = NTOK // CH         # 16
NCHM = NTOKM // CH       # 32
SCALE = float(1.0 / np.sqrt(HD))

_cached = {}


def _build():
    import concourse.bass as bass
    import concourse.mybir as mybir
    import concourse.tile as tile
    from concourse import bacc
    from concourse.masks import make_identity

    f32 = mybir.dt.float32
    f32r = mybir.dt.float32r
    bf16 = mybir.dt.bfloat16
    AF = mybir.ActivationFunctionType
    ALU = mybir.AluOpType

    nc = bacc.Bacc("TRN2", target_bir_lowering=False, debug=False)

    x_d = nc.dram_tensor("x", [T_OUT, HW, D], f32, kind="ExternalInput")
    mem_d = nc.dram_tensor("memory", [T_IN, HW, D], f32, kind="ExternalInput")
    out_d = nc.dram_tensor("out", [T_OUT, HW, D], f32, kind="ExternalOutput")

    WNAMES = [
        ("sa_wq", D, D), ("sa_wk", D, D), ("sa_wv", D, D), ("sa_wo", D, D),
        ("sa_ff_w1", D, FF), ("sa_ff_w2", FF, D),
        ("ca_wq", D, D), ("ca_wk", D, D), ("ca_wv", D, D), ("ca_wo", D, D),
        ("ff_w1", D, FF), ("ff_w2", FF, D),
    ]
    BNAMES = [("sa_bq", D), ("sa_bk", D), ("sa_bv", D), ("sa_bo", D),
              ("sa_ff_b1", FF), ("sa_ff_b2", D),
              ("ca_bq", D), ("ca_bk", D), ("ca_bv", D), ("ca_bo", D),
              ("ff_b1", FF), ("ff_b2", D),
              ("sa_ln1_g", D), ("sa_ln1_b", D), ("sa_ln2_g", D), ("sa_ln2_b", D),
              ("ln1_g", D), ("ln1_b", D), ("ln2_g", D), ("ln2_b", D),
              ("ln3_g", D), ("ln3_b", D)]
    wd = {n: nc.dram_tensor(n, [ki, ko], f32, kind="ExternalInput")
          for n, ki, ko in WNAMES}
    bd = {n: nc.dram_tensor(n, [k], f32, kind="ExternalInput") for n, k in BNAMES}

    with tile.TileContext(nc) as tc:
        with tc.tile_pool(name="dr", bufs=1, space="DRAM") as dr:
            def chunks(tag, n, shape, dtype):
                return [dr.tile(shape, dtype, tag=f"{tag}{i}", name=f"{tag}{i}")
                        for i in range(n)]
            dt = dict(
                X0T=chunks("X0T", NCH, [128, 2, CH], f32r),
                QT=chunks("QT", NCH, [128, 2, CH], bf16),
                KT=chunks("KT", NCH, [128, 2, CH], bf16),
                OT=chunks("OT", NCH, [128, 2, CH], f32r),
                X1T=chunks("X1T", NCH, [128, 2, CH], f32r),
                QT2=chunks("QT2", NCH, [128, 2, CH], bf16),
                KT2=chunks("KT2", NCHM, [128, 2, CH], bf16),
                OT2=chunks("OT2", NCH, [128, 2, CH], f32r),
                OUTT=chunks("OUTT", NCH, [128, 2, CH], f32),
            )
            _emit(nc, tc, bass, mybir, tile, make_identity, f32, f32r, bf16,
                  AF, ALU, x_d, mem_d, out_d, wd, bd, dt)
    nc.compile()
    return nc


def _head_ap(bass_mod, dram_tile, col0, ncols):
    """AP viewing dram fm chunk tile [128, 2, CH] as [32, 8(head), ncols]
    starting at token col col0 (within chunk). head = kt*4 + rowgrp."""
    pstride = dram_tile.ap[0][0]
    nt = dram_tile.shape[2]
    return bass_mod.AP(
        tensor=dram_tile.tensor, offset=dram_tile.offset + col0,
        ap=[[pstride, 32], [nt, 2], [32 * pstride, 4], [1, ncols]])


def _emit(nc, tc, bass, mybir, tile, make_identity, f32, f32r, bf16, AF, ALU,
          x_d, mem_d, out_d, wd, bd, dt):
    from contextlib import ExitStack

    X0T, QT, KT, OT = dt["X0T"], dt["QT"], dt["KT"], dt["OT"]
    X1T, QT2, KT2, OT2, OUTT = (dt["X1T"], dt["QT2"], dt["KT2"],
                                dt["OT2"], dt["OUTT"])

    es = ExitStack()
    cn = es.enter_context(tc.tile_pool(name="consts", bufs=1))
    idf = cn.tile([128, 128], f32, tag="idf")
    make_identity(nc, idf)
    ones128 = cn.tile([128, 1], f32, tag="ones128")
    nc.vector.memset(ones128, 1.0)
    ones128r = cn.tile([128, 1], f32r, tag="ones128r")
    nc.vector.tensor_copy(ones128r, ones128)
    onesrow_r = cn.tile([1, 128], f32r, tag="onesrow_r")
    nc.vector.tensor_copy(onesrow_r, ones128[0:1, :].to_broadcast([1, 128]))
    ones96b = cn.tile([T_OUT, 1], bf16, tag="ones96b")
    nc.vector.tensor_copy(ones96b, ones128[0:T_OUT, :])
    epst = cn.tile([1, 1], f32, tag="epst")
    nc.vector.memset(epst, 1e-5)
    # multiplicative causal mask^T [k, q]: 1 if k<=q else 0; bf16, x4 heads
    maskf = cn.tile([T_OUT, T_OUT], f32, tag="maskf")
    nc.vector.memset(maskf, 1.0)
    nc.gpsimd.affine_select(out=maskf, in_=maskf, compare_op=ALU.is_ge,
                            fill=0.0, base=0, pattern=[[1, T_OUT]],
                            channel_multiplier=-1)
    maskb = cn.tile([T_OUT, T_OUT], bf16, tag="maskb")
    nc.vector.tensor_copy(maskb, maskf)
    mask_rep = bass.AP(tensor=maskb.tensor, offset=maskb.offset,
                       ap=[maskb.ap[0], [0, 4], maskb.ap[1]])

    def load_weight_f32r(pool, name, ki, ko, tag):
        wt = pool.tile([128, ki // 128, ko], f32, tag=tag + "_f")
        nc.sync.dma_start(out=wt, in_=wd[name].ap().rearrange(
            "(kt p) n -> p kt n", p=128))
        wr = pool.tile([128, ki // 128, ko], f32r, tag=tag)
        nc.gpsimd.tensor_copy(wr, wt)
        return wr

    def load_bias(pool, name, k, tag):
        bt = pool.tile([128, k // 128], f32, tag=tag)
        nc.sync.dma_start(out=bt, in_=bd[name].ap().rearrange(
            "(kt p) -> p kt", p=128))
        return bt

    def fm_layernorm(pool, psp, u, gsb, bsb, out_dtype, tag=""):
        """u: sbuf [128, 2, CH] f32r -> normalized tile (out_dtype)."""
        usq = pool.tile([128, 2, CH], f32r, tag="usq" + tag)
        nc.scalar.activation(out=usq, in_=u.bitcast(f32), func=AF.Square)
        ps_s = psp.tile([1, CH], f32, tag="lnsq", bufs=3)
        ps_q = psp.tile([1, CH], f32, tag="lnsq", bufs=3)
        for k in range(2):
            nc.tensor.matmul(ps_s, ones128r, u[:, k, :], start=(k == 0),
                             stop=(k == 1))
        for k in range(2):
            nc.tensor.matmul(ps_q, ones128r, usq[:, k, :], start=(k == 0),
                             stop=(k == 1))
        m = pool.tile([1, CH], f32, tag="lnm" + tag)
        nc.vector.tensor_scalar(out=m, in0=ps_s, scalar1=1.0 / D, scalar2=None,
                                op0=ALU.mult)
        m2 = pool.tile([1, CH], f32, tag="lnm2" + tag)
        nc.vector.tensor_mul(m2, m, m)
        var = pool.tile([1, CH], f32, tag="lnv" + tag)
        nc.vector.scalar_tensor_tensor(out=var, in0=ps_q, scalar=1.0 / D,
                                       in1=m2, op0=ALU.mult, op1=ALU.subtract)
        sd = pool.tile([1, CH], f32, tag="lnsd" + tag)
        nc.scalar.activation(out=sd, in_=var, func=AF.Sqrt, bias=epst)
        rstd = pool.tile([1, CH], f32r, tag="lnr" + tag)
        with nc.allow_low_precision(reason="f32r is full fp32 width"):
            nc.vector.reciprocal(out=rstd, in_=sd)
        c0 = pool.tile([1, CH], f32, tag="lnc0" + tag)
        nc.gpsimd.tensor_mul(c0, m, rstd.bitcast(f32))
        rstd_ps = psp.tile([128, CH], f32, tag="lnrb", bufs=2)
        nc.tensor.matmul(rstd_ps, onesrow_r, rstd, start=True, stop=True)
        c0b = pool.tile([128, CH], f32, tag="lnc0b" + tag)
        nc.gpsimd.partition_broadcast(c0b, c0)
        o = pool.tile([128, 2, CH], out_dtype, tag="lno" + tag)
        for k in range(2):
            t1 = pool.tile([128, CH], f32, tag="lnt1" + tag)
            nc.vector.tensor_tensor(out=t1, in0=u[:, k, :], in1=rstd_ps,
                                    op=ALU.mult)
            nc.gpsimd.tensor_tensor(out=t1, in0=t1, in1=c0b, op=ALU.subtract)
            nc.vector.tensor_scalar(out=o[:, k, :], in0=t1,
                                    scalar1=gsb[:, k:k + 1],
                                    scalar2=bsb[:, k:k + 1],
                                    op0=ALU.mult, op1=ALU.add)
        return o

    # ------- Scope A: x transpose + SA Q/K/V projections + SA attention -----
    # one pool scope, emission interleaved per 4-hw chunk so the scheduler
    # pipelines transposes -> projections -> attention
    with tc.tile_pool(name="scA", bufs=6) as pA, \
         tc.tile_pool(name="scAw", bufs=1) as pw, \
         tc.tile_pool(name="scAps", bufs=2, space="PSUM") as pAps:
        wq = load_weight_f32r(pw, "sa_wq", D, D, "wq")
        wk = load_weight_f32r(pw, "sa_wk", D, D, "wk")
        wv = load_weight_f32r(pw, "sa_wv", D, D, "wv")
        bq = load_bias(pw, "sa_bq", D, "bq")
        bqs = pw.tile([128, 2], f32, tag="bqs")
        nc.vector.tensor_scalar(out=bqs, in0=bq, scalar1=SCALE, scalar2=None,
                                op0=ALU.mult)
        bk = load_bias(pw, "sa_bk", D, "bk")


        for c in range(NCH):
            # transposes for the 4 hw of this chunk
            xch = pA.tile([128, 2, CH], f32r, tag="xch")
            for j in range(4):
                hw = 4 * c + j
                xin = pA.tile([96, D], f32, tag="xin")
                nc.sync.dma_start(out=xin, in_=x_d.ap()[:, hw, :])
                for kt in range(2):
                    tp = pAps.tile([128, 96], f32, tag="pp", bufs=2)
                    nc.tensor.transpose(tp, xin[:, 128 * kt:128 * (kt + 1)],
                                        idf[0:96, 0:96])
                    if (hw + kt) % 2 == 0:
                        nc.scalar.activation(out=xch[:, kt, 96 * j:96 * (j + 1)],
                                             in_=tp, func=AF.Copy)
                    else:
                        nc.vector.tensor_copy(xch[:, kt, 96 * j:96 * (j + 1)], tp)
            nc.sync.dma_start(out=X0T[c], in_=xch)
            # Q/K projections for this chunk (reads xch from SBUF directly)
            for (w, bias, scl, dst) in ((wq, bqs, SCALE, QT), (wk, bk, 1.0, KT)):
                ob = pA.tile([128, 2, CH], bf16, tag="ob")
                for mt in range(2):
                    pp = pAps.tile([128, CH], f32, tag="pp", bufs=2)
                    for kt in range(2):
                        nc.tensor.matmul(pp, w[:, kt, 128 * mt:128 * (mt + 1)],
                                         xch[:, kt, :], start=(kt == 0),
                                         stop=(kt == 1))
                    nc.scalar.activation(out=ob[:, mt, :], in_=pp,
                                         func=AF.Identity, scale=scl,
                                         bias=bias[:, mt:mt + 1])
                nc.sync.dma_start(out=dst[c], in_=ob)
            # V projection per hw (token-major); vb stays in SBUF for attention
            vbs = []
            for j in range(4):
                hw = 4 * c + j
                pv = pAps.tile([96, D], f32, tag="pp", bufs=2)
                for kt in range(2):
                    nc.tensor.matmul(pv, xch[:, kt, 96 * j:96 * (j + 1)],
                                     wv[:, kt, :], start=(kt == 0), stop=(kt == 1))
                vb = pA.tile([96, D], bf16, tag="vb", bufs=8)
                nc.scalar.activation(out=vb, in_=pv, func=AF.Copy)
                vbs.append(vb)
            # SA attention for the 4 hw of this chunk
            for j in range(4):
                hw = 4 * c + j
                q0 = 96 * j
                qh = pA.tile([32, NH, T_OUT], bf16, tag="qh")
                kh = pA.tile([32, NH, T_OUT], bf16, tag="kh")
                vtm = vbs[j]
                nc.sync.dma_start(out=qh, in_=_head_ap(bass, QT[c], q0, T_OUT))
                nc.sync.dma_start(out=kh, in_=_head_ap(bass, KT[c], q0, T_OUT))
                osb = pA.tile([32, 2, 4, T_OUT], f32r, tag="osb")
                for g in range(2):
                    st = pAps.tile([T_OUT, 4, T_OUT], f32, tag="st", bufs=2)
                    for hp in range(4):
                        h = g * 4 + hp
                        nc.tensor.matmul(st[:, hp, :], kh[:, h, :], qh[:, h, :],
                                         start=True, stop=True,
                                         skip_group_check=True)
                    pt = pA.tile([T_OUT, 4, T_OUT], bf16, tag="pt")
                    nc.scalar.activation(out=pt, in_=st, func=AF.Exp)
                    nc.vector.tensor_tensor(out=pt, in0=pt, in1=mask_rep,
                                            op=ALU.mult)
                    dn = pAps.tile([1, 4 * T_OUT], f32, tag="dn", bufs=2)
                    nc.tensor.matmul(dn, ones96b,
                                     pt.rearrange("p a b -> p (a b)"),
                                     start=True, stop=True)
                    rec = pA.tile([1, 4 * T_OUT], f32, tag="rec")
                    nc.vector.reciprocal(out=rec, in_=dn)
                    rb = pA.tile([32, 4, T_OUT], f32, tag="rb")
                    nc.gpsimd.partition_broadcast(
                        rb.rearrange("p a b -> p (a b)"), rec)
                    av = pAps.tile([32, 4, T_OUT], f32, tag="av", bufs=2)
                    for hp in range(4):
                        h = g * 4 + hp
                        nc.tensor.matmul(av[:, hp, :],
                                         vtm[:, 32 * h:32 * h + 32],
                                         pt[:, hp, :], start=True, stop=True,
                                         skip_group_check=True)
                    nc.vector.tensor_tensor(out=osb[:, g, :, :], in0=av,
                                            in1=rb, op=ALU.mult)
                nc.sync.dma_start(out=_head_ap(bass, OT[c], q0, T_OUT), in_=osb)

    # -------- Ph4+5 fused: O-proj + sa_ln1 + FFN + sa_ln2 + (+x0) ln1 -------
    with tc.tile_pool(name="ph4w", bufs=1) as pw, \
         tc.tile_pool(name="ph4", bufs=2) as p4, \
         tc.tile_pool(name="ph4ps", bufs=2, space="PSUM") as p4ps:
        wo = load_weight_f32r(pw, "sa_wo", D, D, "wo")
        bo = load_bias(pw, "sa_bo", D, "bo")
        bvf = load_bias(pw, "sa_bv", D, "bvf")
        bvr = pw.tile([128, 2, 128], f32r, tag="bvr")
        for kt in range(2):
            nc.vector.tensor_copy(bvr[:, kt, :],
                                  bvf[:, kt:kt + 1].to_broadcast([128, 128]))
        bo2 = pw.tile([128, 2], f32, tag="bo2")
        for mt in range(2):
            bps = p4ps.tile([128, 128], f32, tag="pp4", bufs=3)
            for kt in range(2):
                nc.tensor.matmul(bps, wo[:, kt, 128 * mt:128 * (mt + 1)],
                                 bvr[:, kt, :], start=(kt == 0),
                                 stop=(kt == 1))
            nc.vector.tensor_tensor(out=bo2[:, mt:mt + 1], in0=bps[:, 0:1],
                                    in1=bo[:, mt:mt + 1], op=ALU.add)
        g1 = load_bias(pw, "sa_ln1_g", D, "g1")
        b1 = load_bias(pw, "sa_ln1_b", D, "b1")
        w1 = load_weight_f32r(pw, "sa_ff_w1", D, FF, "w1")
        w2 = load_weight_f32r(pw, "sa_ff_w2", FF, D, "w2")
        fb1 = load_bias(pw, "sa_ff_b1", FF, "fb1")
        fb2 = load_bias(pw, "sa_ff_b2", D, "fb2")
        g2 = load_bias(pw, "sa_ln2_g", D, "g2")
        b2 = load_bias(pw, "sa_ln2_b", D, "b2")
        gl1 = load_bias(pw, "ln1_g", D, "gl1")
        bl1 = load_bias(pw, "ln1_b", D, "bl1")
        for c in range(NCH):
            oc = p4.tile([128, 2, CH], f32r, tag="oc")
            nc.sync.dma_start(out=oc, in_=OT[c])
            x0c = p4.tile([128, 2, CH], f32r, tag="x0c")
            nc.sync.dma_start(out=x0c, in_=X0T[c])
            u = p4.tile([128, 2, CH], f32r, tag="u")
            for mt in range(2):
                pp = p4ps.tile([128, CH], f32, tag="pp4", bufs=3)
                for kt in range(2):
                    nc.tensor.matmul(pp, wo[:, kt, 128 * mt:128 * (mt + 1)],
                                     oc[:, kt, :], start=(kt == 0), stop=(kt == 1))
                nc.vector.scalar_tensor_tensor(
                    out=u[:, mt, :], in0=pp, scalar=bo2[:, mt:mt + 1],
                    in1=x0c[:, mt, :].bitcast(f32), op0=ALU.add, op1=ALU.add)
            h1 = fm_layernorm(p4, p4ps, u, g1, b1, f32r, tag="4")
            hh = p4.tile([128, FF // 128, CH], f32r, tag="hh")
            for mt in range(FF // 128):
                pp = p4ps.tile([128, CH], f32, tag="pp4", bufs=3)
                for kt in range(2):
                    nc.tensor.matmul(pp, w1[:, kt, 128 * mt:128 * (mt + 1)],
                                     h1[:, kt, :], start=(kt == 0), stop=(kt == 1))
                nc.scalar.activation(out=hh[:, mt, :], in_=pp, func=AF.Relu,
                                     bias=fb1[:, mt:mt + 1])
            u2 = p4.tile([128, 2, CH], f32r, tag="u2")
            for mt in range(2):
                pp = p4ps.tile([128, CH], f32, tag="pp4", bufs=3)
                for kt in range(FF // 128):
                    nc.tensor.matmul(pp, w2[:, kt, 128 * mt:128 * (mt + 1)],
                                     hh[:, kt, :], start=(kt == 0),
                                     stop=(kt == FF // 128 - 1))
                nc.vector.scalar_tensor_tensor(
                    out=u2[:, mt, :], in0=pp, scalar=fb2[:, mt:mt + 1],
                    in1=h1[:, mt, :].bitcast(f32), op0=ALU.add, op1=ALU.add)
            s2 = fm_layernorm(p4, p4ps, u2, g2, b2, f32r, tag="4b")
            u3 = p4.tile([128, 2, CH], f32r, tag="u3")
            nc.vector.tensor_tensor(out=u3, in0=s2.bitcast(f32),
                                    in1=x0c.bitcast(f32), op=ALU.add)
            x1 = fm_layernorm(p4, p4ps, u3, gl1, bl1, f32r, tag="4c")
            nc.sync.dma_start(out=X1T[c], in_=x1)

    # ------ Scope C: memory transpose + CA projections + CA attention -------
    with tc.tile_pool(name="scC", bufs=6) as pC, \
         tc.tile_pool(name="scCw", bufs=1) as pw, \
         tc.tile_pool(name="scCps", bufs=2, space="PSUM") as pCps:
        cwq = load_weight_f32r(pw, "ca_wq", D, D, "cwq")
        cwk = load_weight_f32r(pw, "ca_wk", D, D, "cwk")
        cwv = load_weight_f32r(pw, "ca_wv", D, D, "cwv")
        cbq = load_bias(pw, "ca_bq", D, "cbq")
        cbqs = pw.tile([128, 2], f32, tag="cbqs")
        nc.vector.tensor_scalar(out=cbqs, in0=cbq, scalar1=SCALE, scalar2=None,
                                op0=ALU.mult)
        cbk = load_bias(pw, "ca_bk", D, "cbk")


        vcas = {}
        for c in range(NCH):  # 4 hw per iteration
            # q-projection for this 4-hw chunk from X1T
            xc = pC.tile([128, 2, CH], f32r, tag="xcq")
            nc.sync.dma_start(out=xc, in_=X1T[c])
            ob = pC.tile([128, 2, CH], bf16, tag="ob6")
            for mt in range(2):
                pp = pCps.tile([128, CH], f32, tag="pp", bufs=2)
                for kt in range(2):
                    nc.tensor.matmul(pp, cwq[:, kt, 128 * mt:128 * (mt + 1)],
                                     xc[:, kt, :], start=(kt == 0), stop=(kt == 1))
                nc.scalar.activation(out=ob[:, mt, :], in_=pp,
                                     func=AF.Identity, scale=SCALE,
                                     bias=cbqs[:, mt:mt + 1])
            nc.sync.dma_start(out=QT2[c], in_=ob)
            # memory transpose + k/v projections for the 2 memory chunks (2hw ea)
            for mchunk in range(2):
                mc = 2 * c + mchunk
                mch = pC.tile([128, 2, CH], f32r, tag="mch")
                for j in range(2):
                    hw = 2 * mc + j
                    for tt in range(2):
                        xin = pC.tile([96, D], f32, tag="xinm")
                        nc.sync.dma_start(
                            out=xin, in_=mem_d.ap()[96 * tt:96 * (tt + 1), hw, :])
                        for kt in range(2):
                            tp = pCps.tile([128, 96], f32, tag="pp", bufs=2)
                            nc.tensor.transpose(
                                tp, xin[:, 128 * kt:128 * (kt + 1)],
                                idf[0:96, 0:96])
                            q = 192 * j + 96 * tt
                            if (hw + tt + kt) % 2 == 0:
                                nc.scalar.activation(out=mch[:, kt, q:q + 96],
                                                     in_=tp, func=AF.Copy)
                            else:
                                nc.vector.tensor_copy(mch[:, kt, q:q + 96], tp)
                ob2 = pC.tile([128, 2, CH], bf16, tag="ob6")
                for mt in range(2):
                    pp = pCps.tile([128, CH], f32, tag="pp", bufs=2)
                    for kt in range(2):
                        nc.tensor.matmul(pp,
                                         cwk[:, kt, 128 * mt:128 * (mt + 1)],
                                         mch[:, kt, :], start=(kt == 0),
                                         stop=(kt == 1))
                    nc.scalar.activation(out=ob2[:, mt, :], in_=pp,
                                         func=AF.Identity,
                                         bias=cbk[:, mt:mt + 1])
                nc.sync.dma_start(out=KT2[mc], in_=ob2)
                for j in range(2):
                    hw = 2 * mc + j
                    for tt in range(2):
                        q = 192 * j + 96 * tt
                        pv = pCps.tile([96, D], f32, tag="pp", bufs=2)
                        for kt in range(2):
                            nc.tensor.matmul(pv, mch[:, kt, q:q + 96],
                                             cwv[:, kt, :], start=(kt == 0),
                                             stop=(kt == 1))
                        vb = pC.tile([96, D], bf16, tag="vb6", bufs=16)
                        nc.scalar.activation(out=vb, in_=pv, func=AF.Copy)
                        vcas[(hw, tt)] = vb
            # CA attention for the 4 hw of this chunk
            for j in range(4):
                hw = 4 * c + j
                q0 = 96 * j
                k0 = 192 * (hw % 2)
                qh = pC.tile([32, NH, T_OUT], bf16, tag="qh7")
                kh = pC.tile([32, NH, T_IN], bf16, tag="kh7")
                v0 = vcas[(hw, 0)]
                v1 = vcas[(hw, 1)]
                nc.sync.dma_start(out=qh, in_=_head_ap(bass, QT2[c], q0, T_OUT))
                nc.sync.dma_start(out=kh,
                                  in_=_head_ap(bass, KT2[hw // 2], k0, T_IN))
                osb = pC.tile([32, 2, 4, T_OUT], f32r, tag="osb7")
                for g in range(2):
                    pts = []
                    dn = pCps.tile([1, 4 * T_OUT], f32, tag="dn7", bufs=2)
                    for tt in range(2):
                        st = pCps.tile([T_OUT, 4, T_OUT], f32, tag="st7", bufs=2)
                        for hp in range(4):
                            h = g * 4 + hp
                            nc.tensor.matmul(st[:, hp, :],
                                             kh[:, h, 96 * tt:96 * (tt + 1)],
                                             qh[:, h, :], start=True, stop=True,
                                             skip_group_check=True)
                        pt = pC.tile([T_OUT, 4, T_OUT], bf16, tag=f"pt7_{tt}")
                        nc.scalar.activation(out=pt, in_=st, func=AF.Exp)
                        pts.append(pt)
                    for tt in range(2):
                        nc.tensor.matmul(dn, ones96b,
                                         pts[tt].rearrange("p a b -> p (a b)"),
                                         start=(tt == 0), stop=(tt == 1))
                    rec = pC.tile([1, 4 * T_OUT], f32, tag="rec7")
                    nc.vector.reciprocal(out=rec, in_=dn)
                    rb = pC.tile([32, 4, T_OUT], f32, tag="rb7")
                    nc.gpsimd.partition_broadcast(
                        rb.rearrange("p a b -> p (a b)"), rec)
                    for hp in range(4):
                        h = g * 4 + hp
                        av = pCps.tile([32, T_OUT], f32, tag="av7", bufs=2)
                        for tt, vv in ((0, v0), (1, v1)):
                            nc.tensor.matmul(av, vv[:, 32 * h:32 * h + 32],
                                             pts[tt][:, hp, :],
                                             start=(tt == 0), stop=(tt == 1))
                        nc.vector.tensor_tensor(out=osb[:, g, hp, :], in0=av,
                                                in1=rb[:, hp, :], op=ALU.mult)
                nc.sync.dma_start(out=_head_ap(bass, OT2[c], q0, T_OUT),
                                  in_=osb)

    # ---------------- Ph8: CA O-proj + ln2 + FFN + ln3 -> OUTT --------------
    with tc.tile_pool(name="ph8w", bufs=1) as pw, \
         tc.tile_pool(name="ph8", bufs=2) as p8, \
         tc.tile_pool(name="ph8ps", bufs=2, space="PSUM") as p8ps:
        cwo = load_weight_f32r(pw, "ca_wo", D, D, "cwo")
        cbo = load_bias(pw, "ca_bo", D, "cbo")
        cbvf = load_bias(pw, "ca_bv", D, "cbvf")
        cbvr = pw.tile([128, 2, 128], f32r, tag="cbvr")
        for kt in range(2):
            nc.vector.tensor_copy(cbvr[:, kt, :],
                                  cbvf[:, kt:kt + 1].to_broadcast([128, 128]))
        cbo2 = pw.tile([128, 2], f32, tag="cbo2")
        for mt in range(2):
            bps = p8ps.tile([128, 128], f32, tag="pp8", bufs=3)
            for kt in range(2):
                nc.tensor.matmul(bps, cwo[:, kt, 128 * mt:128 * (mt + 1)],
                                 cbvr[:, kt, :], start=(kt == 0),
                                 stop=(kt == 1))
            nc.vector.tensor_tensor(out=cbo2[:, mt:mt + 1], in0=bps[:, 0:1],
                                    in1=cbo[:, mt:mt + 1], op=ALU.add)
        gl2 = load_bias(pw, "ln2_g", D, "gl2")
        bl2 = load_bias(pw, "ln2_b", D, "bl2")
        fw1 = load_weight_f32r(pw, "ff_w1", D, FF, "fw1")
        fw2 = load_weight_f32r(pw, "ff_w2", FF, D, "fw2")
        fbb1 = load_bias(pw, "ff_b1", FF, "fbb1")
        fbb2 = load_bias(pw, "ff_b2", D, "fbb2")
        gl3 = load_bias(pw, "ln3_g", D, "gl3")
        bl3 = load_bias(pw, "ln3_b", D, "bl3")
        for c in range(NCH):
            oc = p8.tile([128, 2, CH], f32r, tag="oc8")
            nc.sync.dma_start(out=oc, in_=OT2[c])
            x1c = p8.tile([128, 2, CH], f32r, tag="x1c8")
            nc.sync.dma_start(out=x1c, in_=X1T[c])
            u = p8.tile([128, 2, CH], f32r, tag="u8")
            for mt in range(2):
                pp = p8ps.tile([128, CH], f32, tag="pp8", bufs=3)
                for kt in range(2):
                    nc.tensor.matmul(pp, cwo[:, kt, 128 * mt:128 * (mt + 1)],
                                     oc[:, kt, :], start=(kt == 0), stop=(kt == 1))
                nc.vector.scalar_tensor_tensor(
                    out=u[:, mt, :], in0=pp, scalar=cbo2[:, mt:mt + 1],
                    in1=x1c[:, mt, :].bitcast(f32), op0=ALU.add, op1=ALU.add)
            x2 = fm_layernorm(p8, p8ps, u, gl2, bl2, f32r, tag="8a")
            hh = p8.tile([128, FF // 128, CH], f32r, tag="hh8")
            for mt in range(FF // 128):
                pp = p8ps.tile([128, CH], f32, tag="pp8", bufs=3)
                for kt in range(2):
                    nc.tensor.matmul(pp, fw1[:, kt, 128 * mt:128 * (mt + 1)],
                                     x2[:, kt, :], start=(kt == 0), stop=(kt == 1))
                nc.scalar.activation(out=hh[:, mt, :], in_=pp, func=AF.Relu,
                                     bias=fbb1[:, mt:mt + 1])
            u2 = p8.tile([128, 2, CH], f32r, tag="u28")
            for mt in range(2):
                pp = p8ps.tile([128, CH], f32, tag="pp8", bufs=3)
                for kt in range(FF // 128):
                    nc.tensor.matmul(pp, fw2[:, kt, 128 * mt:128 * (mt + 1)],
                                     hh[:, kt, :], start=(kt == 0),
                                     stop=(kt == FF // 128 - 1))
                nc.vector.scalar_tensor_tensor(
                    out=u2[:, mt, :], in0=pp, scalar=fbb2[:, mt:mt + 1],
                    in1=x2[:, mt, :].bitcast(f32), op0=ALU.add, op1=ALU.add)
            oo = fm_layernorm(p8, p8ps, u2, gl3, bl3, f32, tag="8b")
            nc.sync.dma_start(out=OUTT[c], in_=oo)

    # ---------------- Ph9: transpose back to [T, HW, D] ----------------
    with tc.tile_pool(name="ph9", bufs=4) as p9, \
         tc.tile_pool(name="ph9ps", bufs=4, space="PSUM") as p9ps:
        for hw in range(HW):
            q = 96 * (hw % 4)
            xc = p9.tile([128, 2, 96], f32, tag="xc9")
            nc.sync.dma_start(out=xc, in_=OUTT[hw // 4][:, :, q:q + 96])
            tm = p9.tile([96, D], f32, tag="tm9")
            for kt in range(2):
                tp = p9ps.tile([96, 128], f32, tag="tp9")
                nc.tensor.transpose(tp, xc[:, kt, :], idf)
                if (hw + kt) % 2 == 0:
                    nc.scalar.activation(out=tm[:, 128 * kt:128 * (kt + 1)],
                                         in_=tp, func=AF.Copy)
                else:
                    nc.vector.tensor_copy(tm[:, 128 * kt:128 * (kt + 1)], tp)
            nc.sync.dma_start(out=out_d.ap()[:, hw, :], in_=tm)
    es.close()


def _make_runner(nc):
    """Cached jitted SPMD runner (avoids per-call retracing of
    run_bass_via_pjrt's fresh closures)."""
    import jax
    from jax.sharding import Mesh, PartitionSpec
    from jax.experimental.shard_map import shard_map
    import concourse.mybir as mybir
    from concourse.bass2jax import (_bass_exec_p, install_neuronx_cc_hook,
                                    partition_id_tensor)

    install_neuronx_cc_hook()
    partition_name = (nc.partition_id_tensor.name
                      if nc.partition_id_tensor else None)
    in_names, out_names, out_avals, zero_outs = [], [], [], []
    for alloc in nc.m.functions[0].allocations:
        if not isinstance(alloc, mybir.MemoryLocationSet):
            continue
        name = alloc.memorylocations[0].name
        if alloc.kind == "ExternalInput":
            if name != partition_name:
                in_names.append(name)
        elif alloc.kind == "ExternalOutput":
            shape = tuple(alloc.tensor_shape)
            dtype = mybir.dt.np(alloc.dtype)
            out_names.append(name)
            out_avals.append(jax.core.ShapedArray(shape, dtype))
            zero_outs.append(np.zeros(shape, dtype))
    n_params = len(in_names)
    all_names = list(in_names) + list(out_names)
    if partition_name is not None:
        all_names.append(partition_name)
    donate = tuple(range(n_params, n_params + len(out_names)))

    def _body(*args):
        operands = list(args)
        if partition_name is not None:
            operands.append(partition_id_tensor())
        return tuple(_bass_exec_p.bind(
            *operands, out_avals=tuple(out_avals), in_names=tuple(all_names),
            out_names=tuple(out_names), lowering_input_output_aliases=(),
            sim_require_finite=True, sim_require_nnan=True, nc=nc))

    devices = jax.devices()[:B]
    mesh = Mesh(np.asarray(devices), ("core",))
    in_specs = (PartitionSpec("core"),) * (n_params + len(out_names))
    out_specs = (PartitionSpec("core"),) * len(out_names)
    sharded = jax.jit(shard_map(_body, mesh=mesh, in_specs=in_specs,
                                out_specs=out_specs, check_rep=False),
                      donate_argnums=donate, keep_unused=True)

    def run(in_maps):
        concat_in = [np.concatenate([np.asarray(in_maps[c][nm])
                                     for c in range(B)], axis=0)
                     for nm in in_names]
        concat_zeros = [np.zeros((B * z.shape[0], *z.shape[1:]), z.dtype)
                        for z in zero_outs]
        out_arrs = sharded(*concat_in, *concat_zeros)
        oidx = out_names.index("out")
        a = np.asarray(out_arrs[oidx])
        return a.reshape(B, *out_avals[oidx].shape)

    return run


def kernel(**inputs):
    if "nc" not in _cached:
        _cached["nc"] = _build()
        _cached["run"] = _make_runner(_cached["nc"])
    inp = {k: np.asarray(v, dtype=np.float32) for k, v in inputs.items()}
    shared = {k: v for k, v in inp.items() if k not in ("x", "memory")}
    in_maps = []
    for c in range(B):
        m = dict(shared)
        m["x"] = np.ascontiguousarray(inp["x"][c])
        m["memory"] = np.ascontiguousarray(inp["memory"][c])
        in_maps.append(m)
    out = _cached["run"](in_maps)
    return np.ascontiguousarray(out).astype(np.float32)
